# revision 1
# baseline (speedup 1.0000x reference)
"""Trainium2 Bass kernel for nn_CrossAttentionBlock_78881369358733.

The reference block's attention is degenerate: every query attends to a
single broadcast context token, so softmax over N identical scores is
exactly uniform and the attention output equals `v` for every position.
The whole module collapses to

    ctx   = param_tokens @ Wparam + bparam          # [B, C]
    v     = layernorm(ctx) @ Wkv[:, C:]             # [B, C]
    delta = v @ Wout + bout                         # [B, C]
    out   = img_tokens + delta[:, None, :]          # [B, N, C]

(q, Wq, img layernorm params, and the k-half of Wkv are dead.)

Sharding: pure data parallel over B — core b handles batch b. Each core
computes its own tiny delta vector on-device (PE matmuls + DVE/ACT ops)
and streams img tiles through a broadcast add.

Perf notes (cost-model-driven):
- each dma_start costs ~625ns on the shared HWDGE ring + ~900ns sem
  propagation, so small tensors are host-packed into one SWDGE-loaded
  array and img moves in few large DMAs;
- LN moments are broadcast to all 128 partitions with a ones*(1/C)
  matmul so every later step uses cheap per-partition scalars;
- the delta matmuls run as float32r (full-rate fp32 PE mode), with
  producers rounding into f32r tiles off the critical path;
- bparam rides inside an augmented K=17 first matmul: [Wparam; bparam]^T
  @ [param; 1].
"""

import sys

if "/opt/trn_rl_repo" not in sys.path:
    sys.path.append("/opt/trn_rl_repo")

import numpy as np

B, N, C = 8, 1024, 256
PARAM_DIM = 16
EPS = 1e-5
P = 128
NCORES = 8
USE_F32R = False

# wpk layout [17, 257]: rows 0:16 = Wparam, row 16 = bparam;
#   col 256 = [param_tokens[b]; 1.0]
# vecs2 layout [128, 4]: cols 0:2 = ctx_norm_g, 2:4 = ctx_norm_b,
#   both as [128, 2] partition layout (x[j*128+p] at [p, j])

_BUILT = None


def _patch_tile_tail(tile_mod):
    """The stock TileContext tail emits a drain with one sem-wait per live
    proc (rejected by this walrus: too many sync waits per TPB_CTRL)
    followed by an EVSEM barrier + sem reset that faults the exec unit on
    this runtime. A single drain is sufficient for one-shot NEFF execution:
    semaphores are re-initialized by each nrt_execute."""
    from bass_rust import ScopedClock

    def _drain_only(self, tick_clock, wait_clock):
        drain_inst = self.nc.sync.drain()
        wait_clock.add_sem_waits(
            drain_inst.ins, ScopedClock({None: tick_clock.global_clock})
        )
        popped = self.nc._tile_sem_poison_stack.pop()
        assert popped is self._sem_poison

    tile_mod.TileContext._drain_and_barrier = _drain_only


def _build():
    import concourse.bacc as bacc
    import concourse.tile as tile
    from concourse import mybir

    _patch_tile_tail(tile)

    f32 = mybir.dt.float32
    f32r = mybir.dt.float32r if USE_F32R else mybir.dt.float32
    AF = mybir.ActivationFunctionType
    ALU = mybir.AluOpType

    nc = bacc.Bacc("TRN2", target_bir_lowering=False, debug=False)

    img = nc.dram_tensor("img", [N, C], f32, kind="ExternalInput")
    wpk = nc.dram_tensor("wpk", [PARAM_DIM + 1, C + 1], f32, kind="ExternalInput")
    vecs2 = nc.dram_tensor("vecs2", [P, 4], f32, kind="ExternalInput")
    wv = nc.dram_tensor("wv", [C, C], f32, kind="ExternalInput")
    wout = nc.dram_tensor("wout", [C, C], f32, kind="ExternalInput")
    bout = nc.dram_tensor("bout", [C], f32, kind="ExternalInput")
    out = nc.dram_tensor("out", [N, C], f32, kind="ExternalOutput")

    with tile.TileContext(nc) as tc:
        with (
            tc.tile_pool(name="w", bufs=1) as wp,
            tc.tile_pool(name="io", bufs=1) as iop,
            tc.tile_pool(name="ps", bufs=1, space="PSUM") as pp,
        ):
            # ---- SP ring loads in chain-dependency order; the tiny
            # chain-critical wpk goes first ----
            wpk_sb = wp.tile([PARAM_DIM + 1, C + 1], f32)
            nc.sync.dma_start(wpk_sb[:], wpk.ap())
            bout_sb = wp.tile([1, C], f32)
            nc.sync.dma_start(bout_sb[:], bout.ap().rearrange("(o c) -> o c", o=1))
            # wv_sb[p, j*C + c] = Wv[j*128 + p, c] (one DMA per matrix)
            wv_sb = wp.tile([P, 2 * C], f32)
            nc.sync.dma_start(
                wv_sb[:].rearrange("p (j c) -> p j c", j=2),
                wv.ap().rearrange("(j p) c -> p j c", p=P),
            )
            wo_sb = wp.tile([P, 2 * C], f32)
            nc.sync.dma_start(
                wo_sb[:].rearrange("p (j c) -> p j c", j=2),
                wout.ap().rearrange("(j p) c -> p j c", p=P),
            )
            # gamma/beta via SWDGE (Pool) — off the HWDGE ring; needed late
            vecs2_sb = wp.tile([P, 4], f32)
            nc.gpsimd.dma_start(vecs2_sb[:], vecs2.ap())
            # img: 2 DMAs of 512KB; imgL[L][p, j*C+c] = img[L*512 + j*128 + p, c]
            imgL = []
            for L in range(2):
                t = iop.tile([P, 4 * C], f32, tag=f"img_in{L}", name=f"img_in_{L}")
                nc.sync.dma_start(
                    t[:].rearrange("p (j c) -> p j c", j=4),
                    img.ap()[L * 512 : (L + 1) * 512, :].rearrange(
                        "(j p) c -> p j c", p=P
                    ),
                )
                imgL.append(t)

            # ---- constants (DVE memsets; off the critical path) ----
            ones_1 = wp.tile([1, P], f32)
            nc.vector.memset(ones_1[:], 1.0)
            if USE_F32R:
                ones_1r = wp.tile([1, P], f32r)
                nc.vector.tensor_copy(ones_1r[:], ones_1[:])
            else:
                ones_1r = ones_1
            ones_big = wp.tile([P, P], f32)
            nc.vector.memset(ones_big[:], 1.0)
            invC_mat = wp.tile([P, P], f32)
            nc.vector.memset(invC_mat[:], 1.0 / C)
            eps_col = wp.tile([P, 1], f32)
            nc.vector.memset(eps_col[:], EPS)

            # f32r-rounded copies of the delta-matmul operands (DVE, run as
            # soon as their DMAs land; off the chain's critical path)
            if USE_F32R:
                bout_r = wp.tile([1, C], f32r)
                nc.vector.tensor_copy(bout_r[:], bout_sb[:])
            else:
                bout_r = bout_sb

            # ---- delta chain (partition layout: x[j*128+p] at [p, j]) ----
            # K=17 augmented matmul: ctxT + bparamT directly
            KA = PARAM_DIM + 1
            ctx_ps = pp.tile([P, 2], f32, tag="ctx_ps")
            nc.tensor.matmul(
                ctx_ps[:, 0:1], wpk_sb[0:KA, 0:P], wpk_sb[0:KA, C : C + 1],
                start=True, stop=True,
            )
            nc.tensor.matmul(
                ctx_ps[:, 1:2], wpk_sb[0:KA, P:C], wpk_sb[0:KA, C : C + 1],
                start=True, stop=True,
            )

            # stats_in interleaved: cols (x0, x^2_0, x1, x^2_1)
            stats_in = wp.tile([P, 4], f32)
            nc.vector.tensor_copy(stats_in[:, 0:4:2], ctx_ps[:])
            nc.vector.tensor_mul(
                stats_in[:, 1:4:2], stats_in[:, 0:4:2], stats_in[:, 0:4:2]
            )

            # moments broadcast to ALL partitions: ones(1/C)^T @ stats
            sums_ps = pp.tile([P, 4], f32, tag="sums_ps")
            nc.tensor.matmul(sums_ps[:], invC_mat[:], stats_in[:], start=True, stop=True)
            sums_sb = wp.tile([P, 4], f32)
            nc.vector.tensor_copy(sums_sb[:], sums_ps[:])

            moms = wp.tile([P, 2], f32)  # (mean, E[x^2]) on every partition
            nc.vector.tensor_add(moms[:], sums_sb[:, 0:2], sums_sb[:, 2:4])
            m2 = wp.tile([P, 1], f32)
            nc.vector.tensor_mul(m2[:], moms[:, 0:1], moms[:, 0:1])
            var = wp.tile([P, 1], f32)
            nc.vector.tensor_sub(var[:], moms[:, 1:2], m2[:])
            if USE_F32R:
                # wo_r rounding rides the DVE idle gap while ACT runs Sqrt
                wo_r = wp.tile([P, 2 * C], f32r)
                nc.vector.tensor_copy(wo_r[:], wo_sb[:])
            else:
                wo_r = wo_sb
            sd = wp.tile([P, 1], f32)
            nc.scalar.activation(sd[:], var[:], AF.Sqrt, bias=eps_col[:])
            rstd = wp.tile([P, 1], f32)
            nc.vector.reciprocal(rstd[:], sd[:])
            mrs = wp.tile([P, 1], f32)
            nc.vector.tensor_mul(mrs[:], moms[:, 0:1], rstd[:])

            # cnT = (ctxT * rstd - mean*rstd) * gT + bT
            xn = wp.tile([P, 2], f32)
            nc.vector.tensor_scalar(
                xn[:],
                stats_in[:, 0:4:2],
                rstd[:],
                mrs[:],
                op0=ALU.mult,
                op1=ALU.subtract,
            )
            tmpg = wp.tile([P, 2], f32)
            nc.vector.tensor_mul(tmpg[:], xn[:], vecs2_sb[:, 0:2])
            cnT = wp.tile([P, 2], f32)
            nc.vector.tensor_add(cnT[:], tmpg[:], vecs2_sb[:, 2:4])

            # vT[p, j] = v[j*128+p] = sum_k cn[k] * Wv[k, j*128+p]
            vt_ps = pp.tile([P, 2], f32, tag="vt_ps")
            nc.tensor.matmul(vt_ps[:, 0:1], wv_sb[:, 0:P], cnT[:, 0:1], start=True, stop=False)
            nc.tensor.matmul(vt_ps[:, 0:1], wv_sb[:, C : C + P], cnT[:, 1:2], start=False, stop=True)
            nc.tensor.matmul(vt_ps[:, 1:2], wv_sb[:, P:C], cnT[:, 0:1], start=True, stop=False)
            nc.tensor.matmul(vt_ps[:, 1:2], wv_sb[:, C + P : 2 * C], cnT[:, 1:2], start=False, stop=True)

            # vrep_j[k, m] = v[j*128+k] replicated along free dim (scalar
            # operand read straight from PSUM), rounded to f32r on write
            vrep0 = wp.tile([P, P], f32r)
            nc.vector.tensor_scalar_mul(vrep0[:], ones_big[:], vt_ps[:, 0:1])
            vrep1 = wp.tile([P, P], f32r)
            nc.vector.tensor_scalar_mul(vrep1[:], ones_big[:], vt_ps[:, 1:2])

            # delta[p, c] = sum_k v[k] Wout[k, c] + bout[c]  (all partitions)
            # bias term first (start=True): it only depends on bout, and PE
            # is otherwise idle in that window.
            delta_ps = pp.tile([P, C], f32, tag="delta_ps")
            nc.tensor.matmul(
                delta_ps[:], ones_1r[:], bout_r[:], start=True, stop=False
            )
            nc.tensor.matmul(
                delta_ps[:], vrep0[:], wo_r[:, 0:C], start=False, stop=False
            )
            nc.tensor.matmul(
                delta_ps[:], vrep1[:], wo_r[:, C : 2 * C], start=False, stop=True
            )

            # delta duplicated side by side for [128, 512] adds
            delta2 = wp.tile([P, 2 * C], f32)
            nc.vector.tensor_copy(delta2[:, 0:C], delta_ps[:])
            nc.vector.tensor_copy(delta2[:, C : 2 * C], delta_ps[:])

            # ---- stream: out = img + delta, 4 adds + 4 stores of [128, 512]
            for k in range(4):
                ot = iop.tile([P, 2 * C], f32, tag="img_out", bufs=4, name=f"ot_{k}")
                L, off = k // 2, (k % 2) * 2 * C
                nc.vector.tensor_add(ot[:], imgL[L][:, off : off + 2 * C], delta2[:])
                nc.scalar.dma_start(
                    out.ap()[k * 256 : (k + 1) * 256, :].rearrange(
                        "(j p) c -> p j c", p=P
                    ),
                    ot[:].rearrange("p (j c) -> p j c", j=2),
                )

    nc.compile()
    return nc


def get_nc():
    global _BUILT
    if _BUILT is None:
        _BUILT = _build()
    return _BUILT


def _pack_inputs(inputs):
    f = lambda a: np.ascontiguousarray(np.asarray(a, dtype=np.float32))
    img = f(inputs["img_tokens"])  # [B, N, C]
    param = f(inputs["param_tokens"])  # [B, 16]
    wparam = f(inputs["Wparam"])  # [16, C]
    bparam = f(inputs["bparam"])  # [C]
    gln = f(inputs["ctx_norm_g"])  # [C]
    bln = f(inputs["ctx_norm_b"])  # [C]
    wv = f(np.asarray(inputs["Wkv"], dtype=np.float32)[:, C:])  # [C, C]
    wout = f(inputs["Wout"])  # [C, C]
    bout = f(inputs["bout"])  # [C]

    base = np.zeros((PARAM_DIM + 1, C + 1), np.float32)
    base[0:PARAM_DIM, 0:C] = wparam
    base[PARAM_DIM, 0:C] = bparam
    base[PARAM_DIM, C] = 1.0

    vecs2 = np.empty((P, 4), np.float32)
    vecs2[:, 0:2] = gln.reshape(2, P).T
    vecs2[:, 2:4] = bln.reshape(2, P).T
    vecs2 = np.ascontiguousarray(vecs2)

    in_maps = []
    for b in range(NCORES):
        wpk = base.copy()
        wpk[0:PARAM_DIM, C] = param[b]
        in_maps.append(
            {
                "img": img[b],
                "wpk": np.ascontiguousarray(wpk),
                "vecs2": vecs2,
                "wv": wv,
                "wout": wout,
                "bout": bout,
            }
        )
    return in_maps


def kernel(**inputs):
    from concourse.bass_utils import run_bass_kernel_spmd

    nc = get_nc()
    in_maps = _pack_inputs(inputs)
    res = run_bass_kernel_spmd(nc, in_maps, core_ids=list(range(NCORES)))
    out = np.stack([res.results[b]["out"] for b in range(NCORES)], axis=0)
    return out.astype(np.float32)



# revision 8
# speedup vs baseline: 1.2013x; 1.2013x over previous
"""Trainium2 Bass kernel for nn_CrossAttentionBlock_78881369358733.

The reference block's attention is degenerate: every query attends to a
single broadcast context token, so softmax over N identical scores is
exactly uniform and the attention output equals `v` for every position.
The module collapses to

    ctx   = param_tokens @ Wparam + bparam          # [B, C]
    xn    = (ctx - mean) * rsqrt(var + eps)         # LN core
    delta = xn @ W' + const                         # [B, C]
    out   = img_tokens + delta[:, None, :]          # [B, N, C]

with host-side constant folding of the weight-only products
    W'    = diag(ctx_norm_g) @ Wkv[:, C:] @ Wout
    const = ctx_norm_b @ (Wkv[:, C:] @ Wout) + bout
(q, Wq, img layernorm params, and the k-half of Wkv are dead.)

Sharding: pure data parallel over B - core b handles batch b.

Perf design (cost-model-driven, v2):
- img / weights / output move as bf16: halves all large DMA transfers
  (DMA_ENGINES is the serial resource at ~360 GB/s); rel-err ~2e-3,
  well inside the 2e-2 gate.
- one DMA per tensor; all loads issued back-to-back on the SP queue in
  chain-dependency order (wpk -> wq -> img) so the shared HWDGE ring and
  DMA_ENGINES stay packed and the tiny wpk lands first.
- the LN mean is folded into the first matmul: wpk carries 128 extra
  lhsT columns of Wparam_aug @ 1/C, so ctx, and mean-broadcast come out
  of one PE pass (no separate ones-matmul round trip for the mean).
- chain: PE(ctx+mean) -> DVE(x^2) -> PE(sum/C bcast) -> DVE(var) ->
  ACT(sqrt+eps) -> DVE(recip, mean*rstd, xn, xn-replicate) ->
  PE(delta = bias-mm + 2 bf16 matmuls) -> DVE/ACT(psum->sbuf copies).
- adds in bf16 on DVE ([128,512] chunks, 2x/4x DVE modes), stores split
  across the SP and ACT queues so the two HWDGE slots overlap.
"""

import sys

if "/opt/trn_rl_repo" not in sys.path:
    sys.path.append("/opt/trn_rl_repo")

import numpy as np

B, N, C = 8, 1024, 256
PARAM_DIM = 16
KA = PARAM_DIM + 1  # augmented K (bparam row)
EPS = 1e-5
P = 128
NCORES = 8

# wpk layout [17, 641] f32:
#   cols 0:256   = Wparam_aug (rows 0:16 Wparam, row 16 bparam)
#   cols 256:384 = w_mean replicated 128x (w_mean = Wparam_aug @ ones/C)
#   col  384     = [param_tokens[b]; 1.0]
#   row 0, cols 385:641 = const row (f32; cast to bf16 on device;
#     engine APs must start at partition 0/32/64/96, so not row 16)
WPK_W = 641

_BUILT = None


def _patch_tile_tail(tile_mod):
    """The stock TileContext tail emits a drain with one sem-wait per live
    proc (rejected by this walrus: too many sync waits per TPB_CTRL)
    followed by an EVSEM barrier + sem reset that faults the exec unit on
    this runtime. A single drain is sufficient for one-shot NEFF execution:
    semaphores are re-initialized by each nrt_execute."""
    from bass_rust import ScopedClock

    def _drain_only(self, tick_clock, wait_clock):
        drain_inst = self.nc.sync.drain()
        wait_clock.add_sem_waits(
            drain_inst.ins, ScopedClock({None: tick_clock.global_clock})
        )
        popped = self.nc._tile_sem_poison_stack.pop()
        assert popped is self._sem_poison

    tile_mod.TileContext._drain_and_barrier = _drain_only


def _build():
    import concourse.bacc as bacc
    import concourse.tile as tile
    from concourse import mybir

    _patch_tile_tail(tile)

    f32 = mybir.dt.float32
    bf16 = mybir.dt.bfloat16
    AF = mybir.ActivationFunctionType
    ALU = mybir.AluOpType

    nc = bacc.Bacc("TRN2", target_bir_lowering=False, debug=False)

    img = nc.dram_tensor("img", [N, C], bf16, kind="ExternalInput")
    wpk = nc.dram_tensor("wpk", [KA, WPK_W], f32, kind="ExternalInput")
    wq = nc.dram_tensor("wq", [P, 2 * C], bf16, kind="ExternalInput")
    out = nc.dram_tensor("out", [N, C], bf16, kind="ExternalOutput")

    with tile.TileContext(nc) as tc:
        with (
            tc.tile_pool(name="w", bufs=1) as wp,
            tc.tile_pool(name="io", bufs=1) as iop,
            tc.tile_pool(name="ps", bufs=1, space="PSUM") as pp,
        ):
            # ---- loads, all on the SP ring in dependency order ----
            wpk_sb = wp.tile([KA, WPK_W], f32)
            nc.sync.dma_start(wpk_sb[:], wpk.ap())
            # wq_sb[k, j*256+c] = W'[j*128+k, c]
            wq_sb = wp.tile([P, 2 * C], bf16)
            nc.sync.dma_start(wq_sb[:], wq.ap())
            # img_sb[p, j*256+c] = img[8p+j, c]  (contiguous 4KB runs)
            img_sb = iop.tile([P, 8 * C], bf16, name="img_in")
            nc.sync.dma_start(
                img_sb[:].rearrange("p (j c) -> p j c", j=8),
                img.ap().rearrange("(p j) c -> p j c", p=P),
            )

            # ---- constants (DVE memsets, off the critical path) ----
            ones_1 = wp.tile([1, P], bf16)
            nc.vector.memset(ones_1[:], 1.0)
            ones_big = wp.tile([P, P], bf16)
            nc.vector.memset(ones_big[:], 1.0)
            invC = wp.tile([P, P], f32)
            nc.vector.memset(invC[:], 1.0 / C)
            eps_col = wp.tile([P, 1], f32)
            nc.vector.memset(eps_col[:], EPS)

            # const row -> bf16 (off critical path; feeds the bias matmul)
            const_bf = wp.tile([1, C], bf16)
            nc.vector.tensor_copy(const_bf[:], wpk_sb[0:1, C + P + 1 : WPK_W])

            # ---- delta chain ----
            # bias matmul first: PE is idle and it only needs const_bf
            delta_ps = pp.tile([P, C], f32, tag="delta_ps")
            nc.tensor.matmul(delta_ps[:], ones_1[:], const_bf[:], start=True, stop=False)

            # ctx (cols 0,1) + mean broadcast (col 2) in one PE pass
            ctx_ps = pp.tile([P, 3], f32, tag="ctx_ps")
            prm = wpk_sb[0:KA, C + P : C + P + 1]
            nc.tensor.matmul(ctx_ps[:, 0:1], wpk_sb[0:KA, 0:P], prm, start=True, stop=True)
            nc.tensor.matmul(ctx_ps[:, 1:2], wpk_sb[0:KA, P:C], prm, start=True, stop=True)
            nc.tensor.matmul(ctx_ps[:, 2:3], wpk_sb[0:KA, C : C + P], prm, start=True, stop=True)

            # x^2 -> sbuf (ACT Square: engine ops may read only one
            # non-scalar PSUM operand, and ACT reads PSUM cheaply)
            sq_sb = wp.tile([P, 2], f32)
            nc.scalar.activation(sq_sb[:], ctx_ps[:, 0:2], AF.Square)

            # per-half sums / C, broadcast to all partitions
            msum_ps = pp.tile([P, 2], f32, tag="msum_ps")
            nc.tensor.matmul(msum_ps[:], invC[:], sq_sb[:], start=True, stop=True)

            # var = (msum0 + msum1) - mean^2
            m2 = wp.tile([P, 1], f32)
            nc.scalar.activation(m2[:], ctx_ps[:, 2:3], AF.Square)
            var = wp.tile([P, 1], f32)
            nc.vector.tensor_scalar(
                var[:], msum_ps[:, 0:1], msum_ps[:, 1:2], m2[:],
                op0=ALU.add, op1=ALU.subtract,
            )
            sd = wp.tile([P, 1], f32)
            nc.scalar.activation(sd[:], var[:], AF.Sqrt, bias=eps_col[:])
            rstd = wp.tile([P, 1], f32)
            nc.vector.reciprocal(rstd[:], sd[:])
            mrs = wp.tile([P, 1], f32)
            nc.vector.tensor_scalar_mul(mrs[:], rstd[:], ctx_ps[:, 2:3])

            # xnT[p, j] = (ctx - mean) * rstd (f32: scalar-ptr operands must
            # be f32; the bf16 rounding happens on the xnr outputs)
            xnT = wp.tile([P, 2], f32)
            nc.vector.tensor_scalar(
                xnT[:], ctx_ps[:, 0:2], rstd[:], mrs[:],
                op0=ALU.mult, op1=ALU.subtract,
            )
            # xn replicated along the free dim for the vrep-style matmul
            xnr0 = wp.tile([P, P], bf16)
            nc.vector.tensor_scalar_mul(xnr0[:], ones_big[:], xnT[:, 0:1])
            xnr1 = wp.tile([P, P], bf16)
            nc.vector.tensor_scalar_mul(xnr1[:], ones_big[:], xnT[:, 1:2])

            # delta[m, c] = const[c] + sum_k xn[k] W'[k, c]   (all partitions)
            nc.tensor.matmul(delta_ps[:], xnr0[:], wq_sb[:, 0:C], start=False, stop=False)
            nc.tensor.matmul(delta_ps[:], xnr1[:], wq_sb[:, C : 2 * C], start=False, stop=True)

            # delta duplicated side by side for [128, 512] adds
            delta2 = wp.tile([P, 2 * C], bf16)
            nc.vector.tensor_copy(delta2[:, 0:C], delta_ps[:])
            nc.scalar.activation(delta2[:, C : 2 * C], delta_ps[:], AF.Copy)

            # ---- stream: out = img + delta ----
            ot0 = iop.tile([P, 4 * C], bf16, name="ot0")
            ot1 = iop.tile([P, 4 * C], bf16, name="ot1")
            for k in range(4):
                dst = (ot0 if k < 2 else ot1)[:, (k % 2) * 2 * C : (k % 2 + 1) * 2 * C]
                nc.vector.tensor_add(dst, img_sb[:, k * 2 * C : (k + 1) * 2 * C], delta2[:])
            out_ap = out.ap().rearrange("(p j) c -> p j c", p=P)
            nc.sync.dma_start(
                out_ap[:, 0:4, :], ot0[:].rearrange("p (j c) -> p j c", j=4)
            )
            nc.scalar.dma_start(
                out_ap[:, 4:8, :], ot1[:].rearrange("p (j c) -> p j c", j=4)
            )

    nc.compile()
    return nc


def get_nc():
    global _BUILT
    if _BUILT is None:
        _BUILT = _build()
    return _BUILT


def _pack_inputs(inputs):
    import ml_dtypes

    bf = ml_dtypes.bfloat16
    f = lambda a: np.asarray(a, dtype=np.float32)
    img = f(inputs["img_tokens"])  # [B, N, C]
    param = f(inputs["param_tokens"])  # [B, 16]
    wparam = f(inputs["Wparam"])  # [16, C]
    bparam = f(inputs["bparam"])  # [C]
    gln = f(inputs["ctx_norm_g"])  # [C]
    bln = f(inputs["ctx_norm_b"])  # [C]
    wv = f(inputs["Wkv"])[:, C:]  # [C, C]
    wout = f(inputs["Wout"])  # [C, C]
    bout = f(inputs["bout"])  # [C]

    wvo = wv @ wout  # [C, C]
    wprime = (gln[:, None] * wv) @ wout  # [C, C]
    const = bln @ wvo + bout  # [C]

    wp_aug = np.concatenate([wparam, bparam[None, :]], axis=0)  # [17, C]
    w_mean = wp_aug.sum(axis=1) / C  # [17]

    base = np.zeros((KA, WPK_W), np.float32)
    base[:, 0:C] = wp_aug
    base[:, C : C + P] = w_mean[:, None]
    base[PARAM_DIM, C + P] = 1.0
    base[0, C + P + 1 : WPK_W] = const  # const row lives on partition 0

    wq = np.ascontiguousarray(
        wprime.reshape(2, P, C).transpose(1, 0, 2).reshape(P, 2 * C).astype(bf)
    )
    img_bf = img.astype(bf)

    in_maps = []
    for b in range(NCORES):
        wpk = base.copy()
        wpk[0:PARAM_DIM, C + P] = param[b]
        in_maps.append(
            {
                "img": np.ascontiguousarray(img_bf[b]),
                "wpk": np.ascontiguousarray(wpk),
                "wq": wq,
            }
        )
    return in_maps


def kernel(**inputs):
    from concourse.bass_utils import run_bass_kernel_spmd

    nc = get_nc()
    in_maps = _pack_inputs(inputs)
    res = run_bass_kernel_spmd(nc, in_maps, core_ids=list(range(NCORES)))
    out = np.stack(
        [np.asarray(res.results[b]["out"]) for b in range(NCORES)], axis=0
    )
    return out.astype(np.float32)


# revision 12
# speedup vs baseline: 1.3231x; 1.1014x over previous
"""Trainium2 Bass kernel for nn_CrossAttentionBlock_78881369358733.

The reference block's attention is degenerate: every query attends to a
single broadcast context token, so softmax over N identical scores is
exactly uniform and the attention output equals `v` for every position.
The module collapses to

    ctx   = param_tokens @ Wparam + bparam          # [B, C]
    xn    = (ctx - mean) * rsqrt(var + eps)         # LN core
    delta = xn @ W' + const                         # [B, C]
    out   = img_tokens + delta[:, None, :]          # [B, N, C]

with host-side constant folding of the weight-only products
    W'    = diag(ctx_norm_g) @ Wkv[:, C:] @ Wout
    const = ctx_norm_b @ (Wkv[:, C:] @ Wout) + bout
(q, Wq, img layernorm params, and the k-half of Wkv are dead.)

Sharding: pure data parallel over B - core b handles batch b.

Perf design (cost-model-driven, v2):
- img / weights / output move as bf16: halves all large DMA transfers
  (DMA_ENGINES is the serial resource at ~360 GB/s); rel-err ~2e-3,
  well inside the 2e-2 gate.
- one DMA per tensor; all loads issued back-to-back on the SP queue in
  chain-dependency order (wpk -> wq -> img) so the shared HWDGE ring and
  DMA_ENGINES stay packed and the tiny wpk lands first.
- the LN mean is folded into the first matmul: wpk carries 128 extra
  lhsT columns of Wparam_aug @ 1/C, so ctx, and mean-broadcast come out
  of one PE pass (no separate ones-matmul round trip for the mean).
- chain: PE(ctx+mean) -> DVE(x^2) -> PE(sum/C bcast) -> DVE(var) ->
  ACT(sqrt+eps) -> DVE(recip, mean*rstd, xn, xn-replicate) ->
  PE(delta = bias-mm + 2 bf16 matmuls) -> DVE/ACT(psum->sbuf copies).
- adds in bf16 on DVE ([128,512] chunks, 2x/4x DVE modes), stores split
  across the SP and ACT queues so the two HWDGE slots overlap.
"""

import sys

if "/opt/trn_rl_repo" not in sys.path:
    sys.path.append("/opt/trn_rl_repo")

import numpy as np

B, N, C = 8, 1024, 256
PARAM_DIM = 16
KA = PARAM_DIM + 1  # augmented K (bparam row)
EPS = 1e-5
P = 128
NCORES = 8

# wpk layout [17, 641] f32:
#   cols 0:256   = Wparam_aug (rows 0:16 Wparam, row 16 bparam)
#   cols 256:384 = w_mean replicated 128x (w_mean = Wparam_aug @ ones/C)
#   col  384     = [param_tokens[b]; 1.0]
#   row 0, cols 385:641 = const row (f32; cast to bf16 on device;
#     engine APs must start at partition 0/32/64/96, so not row 16)
WPK_W = 641

_BUILT = None


def _patch_tile_tail(tile_mod):
    """The stock TileContext tail emits a drain with one sem-wait per live
    proc (rejected by this walrus: too many sync waits per TPB_CTRL)
    followed by an EVSEM barrier + sem reset that faults the exec unit on
    this runtime. A single drain is sufficient for one-shot NEFF execution:
    semaphores are re-initialized by each nrt_execute."""
    from bass_rust import ScopedClock

    def _drain_only(self, tick_clock, wait_clock):
        drain_inst = self.nc.sync.drain()
        wait_clock.add_sem_waits(
            drain_inst.ins, ScopedClock({None: tick_clock.global_clock})
        )
        popped = self.nc._tile_sem_poison_stack.pop()
        assert popped is self._sem_poison

    tile_mod.TileContext._drain_and_barrier = _drain_only


def _build():
    import concourse.bacc as bacc
    import concourse.tile as tile
    from concourse import mybir

    _patch_tile_tail(tile)

    f32 = mybir.dt.float32
    bf16 = mybir.dt.bfloat16
    AF = mybir.ActivationFunctionType
    ALU = mybir.AluOpType

    nc = bacc.Bacc("TRN2", target_bir_lowering=False, debug=False)

    img = nc.dram_tensor("img", [N, C], bf16, kind="ExternalInput")
    wpk = nc.dram_tensor("wpk", [KA, WPK_W], f32, kind="ExternalInput")
    wq = nc.dram_tensor("wq", [P, 2 * C], bf16, kind="ExternalInput")
    out = nc.dram_tensor("out", [N, C], bf16, kind="ExternalOutput")

    with tile.TileContext(nc) as tc:
        with (
            tc.tile_pool(name="w", bufs=1) as wp,
            tc.tile_pool(name="io", bufs=1) as iop,
            tc.tile_pool(name="ps", bufs=1, space="PSUM") as pp,
        ):
            # ---- loads, all on the SP ring in dependency order ----
            wpk_sb = wp.tile([KA, WPK_W], f32)
            nc.sync.dma_start(wpk_sb[:], wpk.ap())
            # wq_sb[k, j*256+c] = W'[j*128+k, c]
            wq_sb = wp.tile([P, 2 * C], bf16)
            nc.sync.dma_start(wq_sb[:], wq.ap())
            # img_sb[p, j*256+c] = img[8p+j, c]  (contiguous 4KB runs)
            img_sb = iop.tile([P, 8 * C], bf16, name="img_in")
            nc.sync.dma_start(
                img_sb[:].rearrange("p (j c) -> p j c", j=8),
                img.ap().rearrange("(p j) c -> p j c", p=P),
            )

            # ---- constants (DVE memsets, off the critical path) ----
            ones_1 = wp.tile([1, P], bf16)
            nc.vector.memset(ones_1[:], 1.0)
            ones_big = wp.tile([P, P], bf16)
            nc.vector.memset(ones_big[:], 1.0)
            invC = wp.tile([P, P], f32)
            nc.vector.memset(invC[:], 1.0 / C)
            eps_col = wp.tile([P, 1], f32)
            nc.vector.memset(eps_col[:], EPS)

            # const row -> bf16 (off critical path; feeds the bias matmul)
            const_bf = wp.tile([1, C], bf16)
            nc.vector.tensor_copy(const_bf[:], wpk_sb[0:1, C + P + 1 : WPK_W])

            # ---- delta chain ----
            # bias matmul first: PE is idle and it only needs const_bf
            delta_ps = pp.tile([P, C], f32, tag="delta_ps")
            nc.tensor.matmul(delta_ps[:], ones_1[:], const_bf[:], start=True, stop=False)

            # ctx (cols 0,1) + mean broadcast (col 2) in one PE pass
            ctx_ps = pp.tile([P, 3], f32, tag="ctx_ps")
            prm = wpk_sb[0:KA, C + P : C + P + 1]
            nc.tensor.matmul(ctx_ps[:, 0:1], wpk_sb[0:KA, 0:P], prm, start=True, stop=True)
            nc.tensor.matmul(ctx_ps[:, 1:2], wpk_sb[0:KA, P:C], prm, start=True, stop=True)
            nc.tensor.matmul(ctx_ps[:, 2:3], wpk_sb[0:KA, C : C + P], prm, start=True, stop=True)

            # ctx -> sbuf via ACT Copy (PSUM-friendly, table-free; keeping
            # Square off ACT leaves Sqrt as the only table op so its
            # LoadActFuncSet lands at t~0 instead of mid-chain)
            ctx_sb = wp.tile([P, 3], f32)
            nc.scalar.activation(ctx_sb[:], ctx_ps[:], AF.Copy)

            # x^2 -> sbuf
            sq_sb = wp.tile([P, 2], f32)
            nc.vector.tensor_mul(sq_sb[:], ctx_sb[:, 0:2], ctx_sb[:, 0:2])

            # per-half sums / C, broadcast to all partitions
            msum_ps = pp.tile([P, 2], f32, tag="msum_ps")
            nc.tensor.matmul(msum_ps[:], invC[:], sq_sb[:], start=True, stop=True)

            # var = (msum0 + msum1) - mean^2
            m2 = wp.tile([P, 1], f32)
            nc.vector.tensor_mul(m2[:], ctx_sb[:, 2:3], ctx_sb[:, 2:3])
            var = wp.tile([P, 1], f32)
            nc.vector.tensor_scalar(
                var[:], msum_ps[:, 0:1], msum_ps[:, 1:2], m2[:],
                op0=ALU.add, op1=ALU.subtract,
            )
            sd = wp.tile([P, 1], f32)
            nc.scalar.activation(sd[:], var[:], AF.Sqrt, bias=eps_col[:])
            rstd = wp.tile([P, 1], f32)
            nc.vector.reciprocal(rstd[:], sd[:])
            mrs = wp.tile([P, 1], f32)
            nc.vector.tensor_scalar_mul(mrs[:], rstd[:], ctx_sb[:, 2:3])

            # xnT[p, j] = (ctx - mean) * rstd (f32: scalar-ptr operands must
            # be f32; the bf16 rounding happens on the xnr outputs)
            xnT = wp.tile([P, 2], f32)
            nc.vector.tensor_scalar(
                xnT[:], ctx_sb[:, 0:2], rstd[:], mrs[:],
                op0=ALU.mult, op1=ALU.subtract,
            )
            # xn replicated along the free dim for the vrep-style matmul
            xnr0 = wp.tile([P, P], bf16)
            nc.vector.tensor_scalar_mul(xnr0[:], ones_big[:], xnT[:, 0:1])
            xnr1 = wp.tile([P, P], bf16)
            nc.vector.tensor_scalar_mul(xnr1[:], ones_big[:], xnT[:, 1:2])

            # delta[m, c] = const[c] + sum_k xn[k] W'[k, c]   (all partitions)
            nc.tensor.matmul(delta_ps[:], xnr0[:], wq_sb[:, 0:C], start=False, stop=False)
            nc.tensor.matmul(delta_ps[:], xnr1[:], wq_sb[:, C : 2 * C], start=False, stop=True)

            # single [P, 256] delta in sbuf; adds broadcast it with a
            # 0-stride AP (no duplicated copies)
            delta_sb = wp.tile([P, C], bf16)
            nc.vector.tensor_copy(delta_sb[:], delta_ps[:])
            delta_bc = (
                delta_sb[:].rearrange("p (o c) -> p o c", o=1).broadcast_to([P, 4, C])
            )

            # ---- stream: out = img + delta, two [128, 1024] adds ----
            ot0 = iop.tile([P, 4 * C], bf16, name="ot0")
            ot1 = iop.tile([P, 4 * C], bf16, name="ot1")
            for k, ot in enumerate((ot0, ot1)):
                nc.vector.tensor_add(
                    ot[:].rearrange("p (j c) -> p j c", j=4),
                    img_sb[:, k * 4 * C : (k + 1) * 4 * C].rearrange(
                        "p (j c) -> p j c", j=4
                    ),
                    delta_bc,
                )
            out_ap = out.ap().rearrange("(p j) c -> p j c", p=P)
            nc.sync.dma_start(
                out_ap[:, 0:4, :], ot0[:].rearrange("p (j c) -> p j c", j=4)
            )
            nc.scalar.dma_start(
                out_ap[:, 4:8, :], ot1[:].rearrange("p (j c) -> p j c", j=4)
            )

    nc.compile()
    return nc


def get_nc():
    global _BUILT
    if _BUILT is None:
        _BUILT = _build()
    return _BUILT


def _pack_inputs(inputs):
    import ml_dtypes

    bf = ml_dtypes.bfloat16
    f = lambda a: np.asarray(a, dtype=np.float32)
    img = f(inputs["img_tokens"])  # [B, N, C]
    param = f(inputs["param_tokens"])  # [B, 16]
    wparam = f(inputs["Wparam"])  # [16, C]
    bparam = f(inputs["bparam"])  # [C]
    gln = f(inputs["ctx_norm_g"])  # [C]
    bln = f(inputs["ctx_norm_b"])  # [C]
    wv = f(inputs["Wkv"])[:, C:]  # [C, C]
    wout = f(inputs["Wout"])  # [C, C]
    bout = f(inputs["bout"])  # [C]

    wvo = wv @ wout  # [C, C]
    wprime = (gln[:, None] * wv) @ wout  # [C, C]
    const = bln @ wvo + bout  # [C]

    wp_aug = np.concatenate([wparam, bparam[None, :]], axis=0)  # [17, C]
    w_mean = wp_aug.sum(axis=1) / C  # [17]

    base = np.zeros((KA, WPK_W), np.float32)
    base[:, 0:C] = wp_aug
    base[:, C : C + P] = w_mean[:, None]
    base[PARAM_DIM, C + P] = 1.0
    base[0, C + P + 1 : WPK_W] = const  # const row lives on partition 0

    wq = np.ascontiguousarray(
        wprime.reshape(2, P, C).transpose(1, 0, 2).reshape(P, 2 * C).astype(bf)
    )
    img_bf = img.astype(bf)

    in_maps = []
    for b in range(NCORES):
        wpk = base.copy()
        wpk[0:PARAM_DIM, C + P] = param[b]
        in_maps.append(
            {
                "img": np.ascontiguousarray(img_bf[b]),
                "wpk": np.ascontiguousarray(wpk),
                "wq": wq,
            }
        )
    return in_maps


def kernel(**inputs):
    from concourse.bass_utils import run_bass_kernel_spmd

    nc = get_nc()
    in_maps = _pack_inputs(inputs)
    res = run_bass_kernel_spmd(nc, in_maps, core_ids=list(range(NCORES)))
    out = np.stack(
        [np.asarray(res.results[b]["out"]) for b in range(NCORES)], axis=0
    )
    return out.astype(np.float32)


# revision 14
# speedup vs baseline: 1.4178x; 1.0716x over previous
"""Trainium2 Bass kernel for nn_CrossAttentionBlock_78881369358733.

The reference block's attention is degenerate: every query attends to a
single broadcast context token, so softmax over N identical scores is
exactly uniform and the attention output equals `v` for every position.
The module collapses to

    ctx   = param_tokens @ Wparam + bparam          # [B, C]
    xn    = (ctx - mean) * rsqrt(var + eps)         # LN core
    delta = xn @ W' + const                         # [B, C]
    out   = img_tokens + delta[:, None, :]          # [B, N, C]

with host-side constant folding of the weight-only products
    W'    = diag(ctx_norm_g) @ Wkv[:, C:] @ Wout
    const = ctx_norm_b @ (Wkv[:, C:] @ Wout) + bout
(q, Wq, img layernorm params, and the k-half of Wkv are dead.)

Sharding: pure data parallel over B - core b handles batch b.

Perf design (cost-model-driven, v2):
- img / weights / output move as bf16: halves all large DMA transfers
  (DMA_ENGINES is the serial resource at ~360 GB/s); rel-err ~2e-3,
  well inside the 2e-2 gate.
- one DMA per tensor; all loads issued back-to-back on the SP queue in
  chain-dependency order (wpk -> wq -> img) so the shared HWDGE ring and
  DMA_ENGINES stay packed and the tiny wpk lands first.
- the LN mean is folded into the first matmul: wpk carries 128 extra
  lhsT columns of Wparam_aug @ 1/C, so ctx, and mean-broadcast come out
  of one PE pass (no separate ones-matmul round trip for the mean).
- chain: PE(ctx+mean) -> DVE(x^2) -> PE(sum/C bcast) -> DVE(var) ->
  ACT(sqrt+eps) -> DVE(recip, mean*rstd, xn, xn-replicate) ->
  PE(delta = bias-mm + 2 bf16 matmuls) -> DVE/ACT(psum->sbuf copies).
- adds in bf16 on DVE ([128,512] chunks, 2x/4x DVE modes), stores split
  across the SP and ACT queues so the two HWDGE slots overlap.
"""

import sys

if "/opt/trn_rl_repo" not in sys.path:
    sys.path.append("/opt/trn_rl_repo")

import numpy as np

B, N, C = 8, 1024, 256
PARAM_DIM = 16
KA = PARAM_DIM + 1  # augmented K (bparam row)
EPS = 1e-5
P = 128
NCORES = 8

# wpk layout [17, 641] f32:
#   cols 0:256   = Wparam_aug (rows 0:16 Wparam, row 16 bparam)
#   cols 256:384 = w_mean replicated 128x (w_mean = Wparam_aug @ ones/C)
#   col  384     = [param_tokens[b]; 1.0]
#   row 0, cols 385:641 = const row (f32; cast to bf16 on device;
#     engine APs must start at partition 0/32/64/96, so not row 16)
WPK_W = 641

_BUILT = None


def _patch_tile_tail(tile_mod):
    """The stock TileContext tail emits a drain with one sem-wait per live
    proc (rejected by this walrus: too many sync waits per TPB_CTRL)
    followed by an EVSEM barrier + sem reset that faults the exec unit on
    this runtime. A single drain is sufficient for one-shot NEFF execution:
    semaphores are re-initialized by each nrt_execute."""
    from bass_rust import ScopedClock

    def _drain_only(self, tick_clock, wait_clock):
        drain_inst = self.nc.sync.drain()
        wait_clock.add_sem_waits(
            drain_inst.ins, ScopedClock({None: tick_clock.global_clock})
        )
        popped = self.nc._tile_sem_poison_stack.pop()
        assert popped is self._sem_poison

    tile_mod.TileContext._drain_and_barrier = _drain_only


def _build():
    import concourse.bacc as bacc
    import concourse.tile as tile
    from concourse import mybir

    _patch_tile_tail(tile)

    f32 = mybir.dt.float32
    bf16 = mybir.dt.bfloat16
    AF = mybir.ActivationFunctionType
    ALU = mybir.AluOpType

    nc = bacc.Bacc("TRN2", target_bir_lowering=False, debug=False)

    img = nc.dram_tensor("img", [N, C], bf16, kind="ExternalInput")
    wpk = nc.dram_tensor("wpk", [KA, WPK_W], f32, kind="ExternalInput")
    wq = nc.dram_tensor("wq", [P, 2 * C], bf16, kind="ExternalInput")
    out = nc.dram_tensor("out", [N, C], bf16, kind="ExternalOutput")

    with tile.TileContext(nc) as tc:
        with (
            tc.tile_pool(name="w", bufs=1) as wp,
            tc.tile_pool(name="io", bufs=1) as iop,
            tc.tile_pool(name="ps", bufs=1, space="PSUM") as pp,
        ):
            # ---- loads, all on the SP ring in dependency order ----
            wpk_sb = wp.tile([KA, WPK_W], f32)
            nc.sync.dma_start(wpk_sb[:], wpk.ap())
            # wq_sb[k, j*256+c] = W'[j*128+k, c]
            wq_sb = wp.tile([P, 2 * C], bf16)
            nc.sync.dma_start(wq_sb[:], wq.ap())
            # img_sb[p, j*256+c] = img[8p+j, c]  (contiguous runs); two DMAs
            # so the first half's completion sem fires ~0.7us earlier
            img_sb = iop.tile([P, 8 * C], bf16, name="img_in")
            img_ap = img.ap().rearrange("(p j) c -> p j c", p=P)
            for h in range(2):
                nc.sync.dma_start(
                    img_sb[:, h * 4 * C : (h + 1) * 4 * C].rearrange(
                        "p (j c) -> p j c", j=4
                    ),
                    img_ap[:, h * 4 : (h + 1) * 4, :],
                )

            # ---- constants (DVE memsets, off the critical path) ----
            ones_1 = wp.tile([1, P], bf16)
            nc.vector.memset(ones_1[:], 1.0)
            ones_big = wp.tile([P, P], bf16)
            nc.vector.memset(ones_big[:], 1.0)
            invC = wp.tile([P, P], f32)
            nc.vector.memset(invC[:], 1.0 / C)
            eps_col = wp.tile([P, 1], f32)
            nc.vector.memset(eps_col[:], EPS)
            # dummy Sqrt so its LoadActFuncSet (1283ns) runs now, while ACT
            # is idle, instead of inline before the real Sqrt mid-chain
            warm = wp.tile([P, 1], f32)
            nc.scalar.activation(warm[:], eps_col[:], AF.Sqrt)

            # const row -> bf16 (off critical path; feeds the bias matmul)
            const_bf = wp.tile([1, C], bf16)
            nc.vector.tensor_copy(const_bf[:], wpk_sb[0:1, C + P + 1 : WPK_W])

            # ---- delta chain ----
            # bias matmul first: PE is idle and it only needs const_bf
            delta_ps = pp.tile([P, C], f32, tag="delta_ps")
            nc.tensor.matmul(delta_ps[:], ones_1[:], const_bf[:], start=True, stop=False)

            # ctx (cols 0,1) + mean broadcast (col 2) in one PE pass
            ctx_ps = pp.tile([P, 3], f32, tag="ctx_ps")
            prm = wpk_sb[0:KA, C + P : C + P + 1]
            nc.tensor.matmul(ctx_ps[:, 0:1], wpk_sb[0:KA, 0:P], prm, start=True, stop=True)
            nc.tensor.matmul(ctx_ps[:, 1:2], wpk_sb[0:KA, P:C], prm, start=True, stop=True)
            nc.tensor.matmul(ctx_ps[:, 2:3], wpk_sb[0:KA, C : C + P], prm, start=True, stop=True)

            # ctx -> sbuf via ACT Copy (PSUM-friendly, table-free; keeping
            # Square off ACT leaves Sqrt as the only table op so its
            # LoadActFuncSet lands at t~0 instead of mid-chain)
            ctx_sb = wp.tile([P, 3], f32)
            nc.scalar.activation(ctx_sb[:], ctx_ps[:], AF.Copy)

            # x^2 -> sbuf
            sq_sb = wp.tile([P, 2], f32)
            nc.vector.tensor_mul(sq_sb[:], ctx_sb[:, 0:2], ctx_sb[:, 0:2])

            # per-half sums / C, broadcast to all partitions
            msum_ps = pp.tile([P, 2], f32, tag="msum_ps")
            nc.tensor.matmul(msum_ps[:], invC[:], sq_sb[:], start=True, stop=True)

            # var = (msum0 + msum1) - mean^2
            m2 = wp.tile([P, 1], f32)
            nc.vector.tensor_mul(m2[:], ctx_sb[:, 2:3], ctx_sb[:, 2:3])
            var = wp.tile([P, 1], f32)
            nc.vector.tensor_scalar(
                var[:], msum_ps[:, 0:1], msum_ps[:, 1:2], m2[:],
                op0=ALU.add, op1=ALU.subtract,
            )
            sd = wp.tile([P, 1], f32)
            nc.scalar.activation(sd[:], var[:], AF.Sqrt, bias=eps_col[:])
            rstd = wp.tile([P, 1], f32)
            nc.vector.reciprocal(rstd[:], sd[:])
            mrs = wp.tile([P, 1], f32)
            nc.vector.tensor_scalar_mul(mrs[:], rstd[:], ctx_sb[:, 2:3])

            # xnT[p, j] = (ctx - mean) * rstd (f32: scalar-ptr operands must
            # be f32; the bf16 rounding happens on the xnr outputs)
            xnT = wp.tile([P, 2], f32)
            nc.vector.tensor_scalar(
                xnT[:], ctx_sb[:, 0:2], rstd[:], mrs[:],
                op0=ALU.mult, op1=ALU.subtract,
            )
            # xn replicated along the free dim for the vrep-style matmul
            xnr0 = wp.tile([P, P], bf16)
            nc.vector.tensor_scalar_mul(xnr0[:], ones_big[:], xnT[:, 0:1])
            xnr1 = wp.tile([P, P], bf16)
            nc.vector.tensor_scalar_mul(xnr1[:], ones_big[:], xnT[:, 1:2])

            # delta[m, c] = const[c] + sum_k xn[k] W'[k, c]   (all partitions)
            nc.tensor.matmul(delta_ps[:], xnr0[:], wq_sb[:, 0:C], start=False, stop=False)
            nc.tensor.matmul(delta_ps[:], xnr1[:], wq_sb[:, C : 2 * C], start=False, stop=True)

            # single [P, 256] delta in sbuf; adds broadcast it with a
            # 0-stride AP (no duplicated copies)
            delta_sb = wp.tile([P, C], bf16)
            nc.vector.tensor_copy(delta_sb[:], delta_ps[:])
            delta_bc = (
                delta_sb[:].rearrange("p (o c) -> p o c", o=1).broadcast_to([P, 4, C])
            )

            # ---- stream: out = img + delta, two [128, 1024] adds ----
            ot0 = iop.tile([P, 4 * C], bf16, name="ot0")
            ot1 = iop.tile([P, 4 * C], bf16, name="ot1")
            for k, ot in enumerate((ot0, ot1)):
                nc.vector.tensor_add(
                    ot[:].rearrange("p (j c) -> p j c", j=4),
                    img_sb[:, k * 4 * C : (k + 1) * 4 * C].rearrange(
                        "p (j c) -> p j c", j=4
                    ),
                    delta_bc,
                )
            out_ap = out.ap().rearrange("(p j) c -> p j c", p=P)
            nc.sync.dma_start(
                out_ap[:, 0:4, :], ot0[:].rearrange("p (j c) -> p j c", j=4)
            )
            nc.scalar.dma_start(
                out_ap[:, 4:8, :], ot1[:].rearrange("p (j c) -> p j c", j=4)
            )

    nc.compile()
    return nc


def get_nc():
    global _BUILT
    if _BUILT is None:
        _BUILT = _build()
    return _BUILT


def _pack_inputs(inputs):
    import ml_dtypes

    bf = ml_dtypes.bfloat16
    f = lambda a: np.asarray(a, dtype=np.float32)
    img = f(inputs["img_tokens"])  # [B, N, C]
    param = f(inputs["param_tokens"])  # [B, 16]
    wparam = f(inputs["Wparam"])  # [16, C]
    bparam = f(inputs["bparam"])  # [C]
    gln = f(inputs["ctx_norm_g"])  # [C]
    bln = f(inputs["ctx_norm_b"])  # [C]
    wv = f(inputs["Wkv"])[:, C:]  # [C, C]
    wout = f(inputs["Wout"])  # [C, C]
    bout = f(inputs["bout"])  # [C]

    wvo = wv @ wout  # [C, C]
    wprime = (gln[:, None] * wv) @ wout  # [C, C]
    const = bln @ wvo + bout  # [C]

    wp_aug = np.concatenate([wparam, bparam[None, :]], axis=0)  # [17, C]
    w_mean = wp_aug.sum(axis=1) / C  # [17]

    base = np.zeros((KA, WPK_W), np.float32)
    base[:, 0:C] = wp_aug
    base[:, C : C + P] = w_mean[:, None]
    base[PARAM_DIM, C + P] = 1.0
    base[0, C + P + 1 : WPK_W] = const  # const row lives on partition 0

    wq = np.ascontiguousarray(
        wprime.reshape(2, P, C).transpose(1, 0, 2).reshape(P, 2 * C).astype(bf)
    )
    img_bf = img.astype(bf)

    in_maps = []
    for b in range(NCORES):
        wpk = base.copy()
        wpk[0:PARAM_DIM, C + P] = param[b]
        in_maps.append(
            {
                "img": np.ascontiguousarray(img_bf[b]),
                "wpk": np.ascontiguousarray(wpk),
                "wq": wq,
            }
        )
    return in_maps


def kernel(**inputs):
    from concourse.bass_utils import run_bass_kernel_spmd

    nc = get_nc()
    in_maps = _pack_inputs(inputs)
    res = run_bass_kernel_spmd(nc, in_maps, core_ids=list(range(NCORES)))
    out = np.stack(
        [np.asarray(res.results[b]["out"]) for b in range(NCORES)], axis=0
    )
    return out.astype(np.float32)


# revision 18
# speedup vs baseline: 1.4421x; 1.0171x over previous
"""Trainium2 Bass kernel for nn_CrossAttentionBlock_78881369358733.

The reference block's attention is degenerate: every query attends to a
single broadcast context token, so softmax over N identical scores is
exactly uniform and the attention output equals `v` for every position.
The module collapses to

    ctx   = param_tokens @ Wparam + bparam          # [B, C]
    xn    = (ctx - mean) * rsqrt(var + eps)         # LN core
    delta = xn @ W' + const                         # [B, C]
    out   = img_tokens + delta[:, None, :]          # [B, N, C]

with host-side constant folding of the weight-only products
    W'    = diag(ctx_norm_g) @ Wkv[:, C:] @ Wout
    const = ctx_norm_b @ (Wkv[:, C:] @ Wout) + bout
(q, Wq, img layernorm params, and the k-half of Wkv are dead.)

Sharding: pure data parallel over B - core b handles batch b.

Perf design (cost-model-driven, v2):
- img / weights / output move as bf16: halves all large DMA transfers
  (DMA_ENGINES is the serial resource at ~360 GB/s); rel-err ~2e-3,
  well inside the 2e-2 gate.
- one DMA per tensor; all loads issued back-to-back on the SP queue in
  chain-dependency order (wpk -> wq -> img) so the shared HWDGE ring and
  DMA_ENGINES stay packed and the tiny wpk lands first.
- the LN mean is folded into the first matmul: wpk carries 128 extra
  lhsT columns of Wparam_aug @ 1/C, so ctx, and mean-broadcast come out
  of one PE pass (no separate ones-matmul round trip for the mean).
- chain: PE(ctx+mean) -> DVE(x^2) -> PE(sum/C bcast) -> DVE(var) ->
  ACT(sqrt+eps) -> DVE(recip, mean*rstd, xn, xn-replicate) ->
  PE(delta = bias-mm + 2 bf16 matmuls) -> DVE/ACT(psum->sbuf copies).
- adds in bf16 on DVE ([128,512] chunks, 2x/4x DVE modes), stores split
  across the SP and ACT queues so the two HWDGE slots overlap.
"""

import sys

if "/opt/trn_rl_repo" not in sys.path:
    sys.path.append("/opt/trn_rl_repo")

import numpy as np

B, N, C = 8, 1024, 256
PARAM_DIM = 16
KA = PARAM_DIM + 1  # augmented K (bparam row)
EPS = 1e-5
P = 128
NCORES = 8

# wpk layout [17, 641] f32:
#   cols 0:256   = Wparam_aug (rows 0:16 Wparam, row 16 bparam)
#   cols 256:384 = w_mean replicated 128x (w_mean = Wparam_aug @ ones/C)
#   col  384     = [param_tokens[b]; 1.0]
#   row 0, cols 385:641 = const row (f32; cast to bf16 on device;
#     engine APs must start at partition 0/32/64/96, so not row 16)
WPK_W = 641

_BUILT = None


def _patch_tile_tail(tile_mod):
    """The stock TileContext tail emits a drain with one sem-wait per live
    proc (rejected by this walrus: too many sync waits per TPB_CTRL)
    followed by an EVSEM barrier + sem reset that faults the exec unit on
    this runtime. A single drain is sufficient for one-shot NEFF execution:
    semaphores are re-initialized by each nrt_execute."""
    from bass_rust import ScopedClock

    def _drain_only(self, tick_clock, wait_clock):
        drain_inst = self.nc.sync.drain()
        wait_clock.add_sem_waits(
            drain_inst.ins, ScopedClock({None: tick_clock.global_clock})
        )
        popped = self.nc._tile_sem_poison_stack.pop()
        assert popped is self._sem_poison

    tile_mod.TileContext._drain_and_barrier = _drain_only


def _build():
    import concourse.bacc as bacc
    import concourse.tile as tile
    from concourse import mybir

    _patch_tile_tail(tile)

    f32 = mybir.dt.float32
    bf16 = mybir.dt.bfloat16
    AF = mybir.ActivationFunctionType
    ALU = mybir.AluOpType

    nc = bacc.Bacc("TRN2", target_bir_lowering=False, debug=False)

    img = nc.dram_tensor("img", [N, C], bf16, kind="ExternalInput")
    wpk = nc.dram_tensor("wpk", [KA, WPK_W], f32, kind="ExternalInput")
    wq = nc.dram_tensor("wq", [P, 2 * C], bf16, kind="ExternalInput")
    out = nc.dram_tensor("out", [N, C], bf16, kind="ExternalOutput")

    with tile.TileContext(nc) as tc:
        with (
            tc.tile_pool(name="w", bufs=1) as wp,
            tc.tile_pool(name="io", bufs=1) as iop,
            tc.tile_pool(name="ps", bufs=1, space="PSUM") as pp,
        ):
            # ---- loads, all on the SP ring in dependency order ----
            wpk_sb = wp.tile([KA, WPK_W], f32)
            nc.sync.dma_start(wpk_sb[:], wpk.ap())
            # wq_sb[k, j*256+c] = W'[j*128+k, c]
            wq_sb = wp.tile([P, 2 * C], bf16)
            nc.sync.dma_start(wq_sb[:], wq.ap())
            # img_sb[p, j*256+c] = img[8p+j, c]  (contiguous runs); two DMAs
            # so the first half's completion sem fires ~0.7us earlier
            img_sb = iop.tile([P, 8 * C], bf16, name="img_in")
            img_ap = img.ap().rearrange("(p j) c -> p j c", p=P)
            for h in range(2):
                nc.sync.dma_start(
                    img_sb[:, h * 4 * C : (h + 1) * 4 * C].rearrange(
                        "p (j c) -> p j c", j=4
                    ),
                    img_ap[:, h * 4 : (h + 1) * 4, :],
                )

            # ---- constants (DVE memsets, off the critical path) ----
            ones_1 = wp.tile([1, P], bf16)
            nc.vector.memset(ones_1[:], 1.0)
            ones_big = wp.tile([P, P], bf16)
            nc.vector.memset(ones_big[:], 1.0)
            invC = wp.tile([P, P], f32)
            nc.vector.memset(invC[:], 1.0 / C)
            eps_col = wp.tile([P, 1], f32)
            nc.vector.memset(eps_col[:], EPS)
            # dummy Sqrt so its LoadActFuncSet (1283ns) runs now, while ACT
            # is idle, instead of inline before the real Sqrt mid-chain
            warm = wp.tile([P, 1], f32)
            nc.scalar.activation(warm[:], eps_col[:], AF.Sqrt)

            # const row -> bf16 (off critical path; feeds the bias matmul)
            const_bf = wp.tile([1, C], bf16)
            nc.vector.tensor_copy(const_bf[:], wpk_sb[0:1, C + P + 1 : WPK_W])

            # ---- delta chain ----
            # bias matmul first: PE is idle and it only needs const_bf
            delta_ps = pp.tile([P, C], f32, tag="delta_ps")
            nc.tensor.matmul(delta_ps[:], ones_1[:], const_bf[:], start=True, stop=False)

            # ctx (cols 0,1) + mean broadcast (col 2) in one PE pass
            ctx_ps = pp.tile([P, 3], f32, tag="ctx_ps")
            prm = wpk_sb[0:KA, C + P : C + P + 1]
            nc.tensor.matmul(ctx_ps[:, 0:1], wpk_sb[0:KA, 0:P], prm, start=True, stop=True)
            nc.tensor.matmul(ctx_ps[:, 1:2], wpk_sb[0:KA, P:C], prm, start=True, stop=True)
            nc.tensor.matmul(ctx_ps[:, 2:3], wpk_sb[0:KA, C : C + P], prm, start=True, stop=True)

            # x^2 -> sbuf. tensor_scalar with a per-partition scalar PTR:
            # scalar operands are exempt from the one-PSUM-operand rule, so
            # ctx never needs a PSUM->SBUF staging copy.
            sq_sb = wp.tile([P, 2], f32)
            nc.vector.tensor_scalar_mul(sq_sb[:, 0:1], ctx_ps[:, 0:1], ctx_ps[:, 0:1])
            nc.vector.tensor_scalar_mul(sq_sb[:, 1:2], ctx_ps[:, 1:2], ctx_ps[:, 1:2])

            # per-half sums / C, broadcast to all partitions
            msum_ps = pp.tile([P, 2], f32, tag="msum_ps")
            nc.tensor.matmul(msum_ps[:], invC[:], sq_sb[:], start=True, stop=True)

            # var = (msum0 + msum1) - mean^2
            m2 = wp.tile([P, 1], f32)
            nc.vector.tensor_scalar_mul(m2[:], ctx_ps[:, 2:3], ctx_ps[:, 2:3])
            var = wp.tile([P, 1], f32)
            nc.vector.tensor_scalar(
                var[:], msum_ps[:, 0:1], msum_ps[:, 1:2], m2[:],
                op0=ALU.add, op1=ALU.subtract,
            )
            sd = wp.tile([P, 1], f32)
            nc.scalar.activation(sd[:], var[:], AF.Sqrt, bias=eps_col[:])
            rstd = wp.tile([P, 1], f32)
            nc.vector.reciprocal(rstd[:], sd[:])
            mrs = wp.tile([P, 1], f32)
            nc.vector.tensor_scalar_mul(mrs[:], rstd[:], ctx_ps[:, 2:3])

            # xnT[p, j] = (ctx - mean) * rstd (f32: scalar-ptr operands must
            # be f32; the bf16 rounding happens on the xnr outputs)
            xnT = wp.tile([P, 2], f32)
            nc.vector.tensor_scalar(
                xnT[:], ctx_ps[:, 0:2], rstd[:], mrs[:],
                op0=ALU.mult, op1=ALU.subtract,
            )
            # xn replicated along the free dim for the vrep-style matmul
            xnr0 = wp.tile([P, P], bf16)
            nc.vector.tensor_scalar_mul(xnr0[:], ones_big[:], xnT[:, 0:1])
            xnr1 = wp.tile([P, P], bf16)
            nc.vector.tensor_scalar_mul(xnr1[:], ones_big[:], xnT[:, 1:2])

            # delta[m, c] = const[c] + sum_k xn[k] W'[k, c]   (all partitions)
            nc.tensor.matmul(delta_ps[:], xnr0[:], wq_sb[:, 0:C], start=False, stop=False)
            nc.tensor.matmul(delta_ps[:], xnr1[:], wq_sb[:, C : 2 * C], start=False, stop=True)

            # single [P, 256] delta in sbuf; adds broadcast it with a
            # 0-stride AP (no duplicated copies)
            delta_sb = wp.tile([P, C], bf16)
            nc.vector.tensor_copy(delta_sb[:], delta_ps[:])
            delta_bc = (
                delta_sb[:].rearrange("p (o c) -> p o c", o=1).broadcast_to([P, 4, C])
            )

            # ---- stream: out = img + delta, two [128, 1024] adds ----
            ot0 = iop.tile([P, 4 * C], bf16, name="ot0")
            ot1 = iop.tile([P, 4 * C], bf16, name="ot1")
            for k, ot in enumerate((ot0, ot1)):
                nc.vector.tensor_add(
                    ot[:].rearrange("p (j c) -> p j c", j=4),
                    img_sb[:, k * 4 * C : (k + 1) * 4 * C].rearrange(
                        "p (j c) -> p j c", j=4
                    ),
                    delta_bc,
                )
            out_ap = out.ap().rearrange("(p j) c -> p j c", p=P)
            # both stores on SP: its HWDGE slot (625) + DGE delay (650) beat
            # ACT's 632+784, and the transfers serialize on DMA_ENGINES anyway
            nc.sync.dma_start(
                out_ap[:, 0:4, :], ot0[:].rearrange("p (j c) -> p j c", j=4)
            )
            nc.sync.dma_start(
                out_ap[:, 4:8, :], ot1[:].rearrange("p (j c) -> p j c", j=4)
            )

    nc.compile()
    return nc


def get_nc():
    global _BUILT
    if _BUILT is None:
        _BUILT = _build()
    return _BUILT


def _pack_inputs(inputs):
    import ml_dtypes

    bf = ml_dtypes.bfloat16
    f = lambda a: np.asarray(a, dtype=np.float32)
    img = f(inputs["img_tokens"])  # [B, N, C]
    param = f(inputs["param_tokens"])  # [B, 16]
    wparam = f(inputs["Wparam"])  # [16, C]
    bparam = f(inputs["bparam"])  # [C]
    gln = f(inputs["ctx_norm_g"])  # [C]
    bln = f(inputs["ctx_norm_b"])  # [C]
    wv = f(inputs["Wkv"])[:, C:]  # [C, C]
    wout = f(inputs["Wout"])  # [C, C]
    bout = f(inputs["bout"])  # [C]

    wvo = wv @ wout  # [C, C]
    wprime = (gln[:, None] * wv) @ wout  # [C, C]
    const = bln @ wvo + bout  # [C]

    wp_aug = np.concatenate([wparam, bparam[None, :]], axis=0)  # [17, C]
    w_mean = wp_aug.sum(axis=1) / C  # [17]

    base = np.zeros((KA, WPK_W), np.float32)
    base[:, 0:C] = wp_aug
    base[:, C : C + P] = w_mean[:, None]
    base[PARAM_DIM, C + P] = 1.0
    base[0, C + P + 1 : WPK_W] = const  # const row lives on partition 0

    wq = np.ascontiguousarray(
        wprime.reshape(2, P, C).transpose(1, 0, 2).reshape(P, 2 * C).astype(bf)
    )
    img_bf = img.astype(bf)

    in_maps = []
    for b in range(NCORES):
        wpk = base.copy()
        wpk[0:PARAM_DIM, C + P] = param[b]
        in_maps.append(
            {
                "img": np.ascontiguousarray(img_bf[b]),
                "wpk": np.ascontiguousarray(wpk),
                "wq": wq,
            }
        )
    return in_maps


def kernel(**inputs):
    from concourse.bass_utils import run_bass_kernel_spmd

    nc = get_nc()
    in_maps = _pack_inputs(inputs)
    res = run_bass_kernel_spmd(nc, in_maps, core_ids=list(range(NCORES)))
    out = np.stack(
        [np.asarray(res.results[b]["out"]) for b in range(NCORES)], axis=0
    )
    return out.astype(np.float32)


# revision 21
# speedup vs baseline: 1.4474x; 1.0037x over previous
"""Trainium2 Bass kernel for nn_CrossAttentionBlock_78881369358733.

The reference block's attention is degenerate: every query attends to a
single broadcast context token, so softmax over N identical scores is
exactly uniform and the attention output equals `v` for every position.
The module collapses to

    ctx   = param_tokens @ Wparam + bparam          # [B, C]
    xn    = (ctx - mean) * rsqrt(var + eps)         # LN core
    delta = xn @ W' + const                         # [B, C]
    out   = img_tokens + delta[:, None, :]          # [B, N, C]

with host-side constant folding of the weight-only products
    W'    = diag(ctx_norm_g) @ Wkv[:, C:] @ Wout
    const = ctx_norm_b @ (Wkv[:, C:] @ Wout) + bout
(q, Wq, img layernorm params, and the k-half of Wkv are dead.)

Sharding: pure data parallel over B - core b handles batch b.

Perf design (cost-model-driven, v2):
- img / weights / output move as bf16: halves all large DMA transfers
  (DMA_ENGINES is the serial resource at ~360 GB/s); rel-err ~2e-3,
  well inside the 2e-2 gate.
- one DMA per tensor; all loads issued back-to-back on the SP queue in
  chain-dependency order (wpk -> wq -> img) so the shared HWDGE ring and
  DMA_ENGINES stay packed and the tiny wpk lands first.
- the LN mean is folded into the first matmul: wpk carries 128 extra
  lhsT columns of Wparam_aug @ 1/C, so ctx, and mean-broadcast come out
  of one PE pass (no separate ones-matmul round trip for the mean).
- chain: PE(ctx+mean) -> DVE(x^2) -> PE(sum/C bcast) -> DVE(var) ->
  ACT(sqrt+eps) -> DVE(recip, mean*rstd, xn, xn-replicate) ->
  PE(delta = bias-mm + 2 bf16 matmuls) -> DVE/ACT(psum->sbuf copies).
- adds in bf16 on DVE ([128,512] chunks, 2x/4x DVE modes), stores split
  across the SP and ACT queues so the two HWDGE slots overlap.
"""

import sys

if "/opt/trn_rl_repo" not in sys.path:
    sys.path.append("/opt/trn_rl_repo")

import numpy as np

B, N, C = 8, 1024, 256
PARAM_DIM = 16
KA = PARAM_DIM + 1  # augmented K (bparam row)
EPS = 1e-5
P = 128
NCORES = 8

# wpk layout [17, 641] f32:
#   cols 0:256   = Wparam_aug (rows 0:16 Wparam, row 16 bparam)
#   cols 256:384 = w_mean replicated 128x (w_mean = Wparam_aug @ ones/C)
#   col  384     = [param_tokens[b]; 1.0]
#   row 0, cols 385:641 = const row (f32; cast to bf16 on device;
#     engine APs must start at partition 0/32/64/96, so not row 16)
WPK_W = 641

_BUILT = None


def _patch_tile_tail(tile_mod):
    """The stock TileContext tail emits a drain with one sem-wait per live
    proc (rejected by this walrus: too many sync waits per TPB_CTRL)
    followed by an EVSEM barrier + sem reset that faults the exec unit on
    this runtime. A single drain is sufficient for one-shot NEFF execution:
    semaphores are re-initialized by each nrt_execute."""
    from bass_rust import ScopedClock

    def _drain_only(self, tick_clock, wait_clock):
        drain_inst = self.nc.sync.drain()
        wait_clock.add_sem_waits(
            drain_inst.ins, ScopedClock({None: tick_clock.global_clock})
        )
        popped = self.nc._tile_sem_poison_stack.pop()
        assert popped is self._sem_poison

    tile_mod.TileContext._drain_and_barrier = _drain_only


def _build():
    import concourse.bacc as bacc
    import concourse.tile as tile
    from concourse import mybir

    _patch_tile_tail(tile)

    f32 = mybir.dt.float32
    bf16 = mybir.dt.bfloat16
    AF = mybir.ActivationFunctionType
    ALU = mybir.AluOpType

    nc = bacc.Bacc("TRN2", target_bir_lowering=False, debug=False)

    img = nc.dram_tensor("img", [N, C], bf16, kind="ExternalInput")
    wpk = nc.dram_tensor("wpk", [KA, WPK_W], f32, kind="ExternalInput")
    wq = nc.dram_tensor("wq", [P, 2 * C], bf16, kind="ExternalInput")
    out = nc.dram_tensor("out", [N, C], bf16, kind="ExternalOutput")

    with tile.TileContext(nc) as tc:
        with (
            tc.tile_pool(name="w", bufs=1) as wp,
            tc.tile_pool(name="io", bufs=1) as iop,
            tc.tile_pool(name="ps", bufs=1, space="PSUM") as pp,
        ):
            # ---- loads, all on the SP ring in dependency order ----
            wpk_sb = wp.tile([KA, WPK_W], f32)
            nc.sync.dma_start(wpk_sb[:], wpk.ap())
            # wq_sb[k, j*256+c] = W'[j*128+k, c]
            wq_sb = wp.tile([P, 2 * C], bf16)
            nc.sync.dma_start(wq_sb[:], wq.ap())
            # img_sb[p, j*256+c] = img[8p+j, c]  (contiguous runs); two DMAs
            # so the first half's completion sem fires ~0.7us earlier
            img_sb = iop.tile([P, 8 * C], bf16, name="img_in")
            img_ap = img.ap().rearrange("(p j) c -> p j c", p=P)
            for h in range(2):
                nc.sync.dma_start(
                    img_sb[:, h * 4 * C : (h + 1) * 4 * C].rearrange(
                        "p (j c) -> p j c", j=4
                    ),
                    img_ap[:, h * 4 : (h + 1) * 4, :],
                )

            # ---- constants (DVE memsets, off the critical path) ----
            ones_1 = wp.tile([1, P], bf16)
            nc.vector.memset(ones_1[:], 1.0)
            ones_big = wp.tile([P, P], bf16)
            nc.vector.memset(ones_big[:], 1.0)
            invC = wp.tile([P, P], f32)
            nc.vector.memset(invC[:], 1.0 / C)
            eps_col = wp.tile([P, 1], f32)
            nc.vector.memset(eps_col[:], EPS)
            # dummy Sqrt so its LoadActFuncSet (1283ns) runs now, while ACT
            # is idle, instead of inline before the real Sqrt mid-chain
            warm = wp.tile([P, 1], f32)
            nc.scalar.activation(warm[:], eps_col[:], AF.Sqrt)

            # const row -> bf16 (off critical path; feeds the bias matmul)
            const_bf = wp.tile([1, C], bf16)
            nc.vector.tensor_copy(const_bf[:], wpk_sb[0:1, C + P + 1 : WPK_W])

            # ---- delta chain ----
            # ctx (cols 0,1) + mean broadcast (col 2) in one PE pass
            ctx_ps = pp.tile([P, 3], f32, tag="ctx_ps")
            prm = wpk_sb[0:KA, C + P : C + P + 1]
            nc.tensor.matmul(ctx_ps[:, 0:1], wpk_sb[0:KA, 0:P], prm, start=True, stop=True)
            nc.tensor.matmul(ctx_ps[:, 1:2], wpk_sb[0:KA, P:C], prm, start=True, stop=True)
            nc.tensor.matmul(ctx_ps[:, 2:3], wpk_sb[0:KA, C : C + P], prm, start=True, stop=True)

            # x^2 -> sbuf. tensor_scalar with a per-partition scalar PTR:
            # scalar operands are exempt from the one-PSUM-operand rule, so
            # ctx never needs a PSUM->SBUF staging copy.
            sq_sb = wp.tile([P, 2], f32)
            nc.vector.tensor_scalar_mul(sq_sb[:, 0:1], ctx_ps[:, 0:1], ctx_ps[:, 0:1])
            nc.vector.tensor_scalar_mul(sq_sb[:, 1:2], ctx_ps[:, 1:2], ctx_ps[:, 1:2])

            # per-half sums / C, broadcast to all partitions
            msum_ps = pp.tile([P, 2], f32, tag="msum_ps")
            nc.tensor.matmul(msum_ps[:], invC[:], sq_sb[:], start=True, stop=True)

            # var = (msum0 + msum1) - mean^2
            m2 = wp.tile([P, 1], f32)
            nc.vector.tensor_scalar_mul(m2[:], ctx_ps[:, 2:3], ctx_ps[:, 2:3])
            var = wp.tile([P, 1], f32)
            nc.vector.tensor_scalar(
                var[:], msum_ps[:, 0:1], msum_ps[:, 1:2], m2[:],
                op0=ALU.add, op1=ALU.subtract,
            )
            sd = wp.tile([P, 1], f32)
            nc.scalar.activation(sd[:], var[:], AF.Sqrt, bias=eps_col[:])
            rstd = wp.tile([P, 1], f32)
            nc.vector.reciprocal(rstd[:], sd[:])

            # xnT[p, j] = (ctx - mean) * rstd in one op (f32: scalar-ptr
            # operands must be f32; bf16 rounding happens on the xnr outputs)
            xnT = wp.tile([P, 2], f32)
            nc.vector.tensor_scalar(
                xnT[:], ctx_ps[:, 0:2], ctx_ps[:, 2:3], rstd[:],
                op0=ALU.subtract, op1=ALU.mult,
            )
            # xn replicated along the free dim for the vrep-style matmul
            xnr0 = wp.tile([P, P], bf16)
            nc.vector.tensor_scalar_mul(xnr0[:], ones_big[:], xnT[:, 0:1])
            xnr1 = wp.tile([P, P], bf16)
            nc.vector.tensor_scalar_mul(xnr1[:], ones_big[:], xnT[:, 1:2])

            # delta[m, c] = const[c] + sum_k xn[k] W'[k, c]   (all partitions)
            # bias matmul placed here in PE program order so it can't delay
            # the sums matmul; it still runs in PE's idle window (it only
            # waits on const_bf)
            delta_ps = pp.tile([P, C], f32, tag="delta_ps")
            nc.tensor.matmul(delta_ps[:], ones_1[:], const_bf[:], start=True, stop=False)
            nc.tensor.matmul(delta_ps[:], xnr0[:], wq_sb[:, 0:C], start=False, stop=False)
            nc.tensor.matmul(delta_ps[:], xnr1[:], wq_sb[:, C : 2 * C], start=False, stop=True)

            # single [P, 256] delta in sbuf; adds broadcast it with a
            # 0-stride AP (no duplicated copies)
            delta_sb = wp.tile([P, C], bf16)
            nc.vector.tensor_copy(delta_sb[:], delta_ps[:])
            delta_bc = (
                delta_sb[:].rearrange("p (o c) -> p o c", o=1).broadcast_to([P, 4, C])
            )

            # ---- stream: out = img + delta, two [128, 1024] adds ----
            ot0 = iop.tile([P, 4 * C], bf16, name="ot0")
            ot1 = iop.tile([P, 4 * C], bf16, name="ot1")
            for k, ot in enumerate((ot0, ot1)):
                nc.vector.tensor_add(
                    ot[:].rearrange("p (j c) -> p j c", j=4),
                    img_sb[:, k * 4 * C : (k + 1) * 4 * C].rearrange(
                        "p (j c) -> p j c", j=4
                    ),
                    delta_bc,
                )
            out_ap = out.ap().rearrange("(p j) c -> p j c", p=P)
            # both stores on SP: its HWDGE slot (625) + DGE delay (650) beat
            # ACT's 632+784, and the transfers serialize on DMA_ENGINES anyway
            nc.sync.dma_start(
                out_ap[:, 0:4, :], ot0[:].rearrange("p (j c) -> p j c", j=4)
            )
            nc.sync.dma_start(
                out_ap[:, 4:8, :], ot1[:].rearrange("p (j c) -> p j c", j=4)
            )

    nc.compile()
    return nc


def get_nc():
    global _BUILT
    if _BUILT is None:
        _BUILT = _build()
    return _BUILT


def _pack_inputs(inputs):
    import ml_dtypes

    bf = ml_dtypes.bfloat16
    f = lambda a: np.asarray(a, dtype=np.float32)
    img = f(inputs["img_tokens"])  # [B, N, C]
    param = f(inputs["param_tokens"])  # [B, 16]
    wparam = f(inputs["Wparam"])  # [16, C]
    bparam = f(inputs["bparam"])  # [C]
    gln = f(inputs["ctx_norm_g"])  # [C]
    bln = f(inputs["ctx_norm_b"])  # [C]
    wv = f(inputs["Wkv"])[:, C:]  # [C, C]
    wout = f(inputs["Wout"])  # [C, C]
    bout = f(inputs["bout"])  # [C]

    wvo = wv @ wout  # [C, C]
    wprime = (gln[:, None] * wv) @ wout  # [C, C]
    const = bln @ wvo + bout  # [C]

    wp_aug = np.concatenate([wparam, bparam[None, :]], axis=0)  # [17, C]
    w_mean = wp_aug.sum(axis=1) / C  # [17]

    base = np.zeros((KA, WPK_W), np.float32)
    base[:, 0:C] = wp_aug
    base[:, C : C + P] = w_mean[:, None]
    base[PARAM_DIM, C + P] = 1.0
    base[0, C + P + 1 : WPK_W] = const  # const row lives on partition 0

    wq = np.ascontiguousarray(
        wprime.reshape(2, P, C).transpose(1, 0, 2).reshape(P, 2 * C).astype(bf)
    )
    img_bf = img.astype(bf)

    in_maps = []
    for b in range(NCORES):
        wpk = base.copy()
        wpk[0:PARAM_DIM, C + P] = param[b]
        in_maps.append(
            {
                "img": np.ascontiguousarray(img_bf[b]),
                "wpk": np.ascontiguousarray(wpk),
                "wq": wq,
            }
        )
    return in_maps


def kernel(**inputs):
    from concourse.bass_utils import run_bass_kernel_spmd

    nc = get_nc()
    in_maps = _pack_inputs(inputs)
    res = run_bass_kernel_spmd(nc, in_maps, core_ids=list(range(NCORES)))
    out = np.stack(
        [np.asarray(res.results[b]["out"]) for b in range(NCORES)], axis=0
    )
    return out.astype(np.float32)
